# revision 1
# baseline (speedup 1.0000x reference)
"""Trainium2 Bass kernel for the nn_LSTMCell problem.

Strategy: data-parallel over the batch dim (4096 -> 8 cores x 512), weights
replicated. All on-chip compute happens in "transposed" orientation
(hidden on PSUM partitions, batch on the free dim) so every matmul operand
can be DMA'd in its natural, contiguous layout:

    gate.T[h, b] = sum_k W.T[k, h] * act.T[k, b]
    matmul(out[M=h128, N=b512], lhsT=WT_tile[K=k128, M=h128], rhs=actT[K=k128, N=b512])

The host pre-packs (transposes + casts to bf16) activations and weights;
only device execution is the measured kernel. Matmuls run in bf16 with fp32
PSUM accumulation; all elementwise math and outputs are fp32.

Per core:
  phase 1: for each of 16 h-tiles: i/f/g gate matmuls (112 MMs), sigmoid/tanh,
           c1 = f*c0 + i*tanh(g)  -> c1 (fp32, kept in SBUF + DMA'd out),
           c1 cast to bf16 (matmul operand for the o gate).
  phase 2: for each of 16 h-tiles: o gate matmuls (48 MMs, incl. W_co @ c1.T),
           o = sigmoid(...), h1 = o * tanh(c1), DMA out.
"""

import numpy as np
import ml_dtypes
from contextlib import ExitStack

BF = ml_dtypes.bfloat16

N_CORES = 8
P = 128          # partition dim / k-tile size / m-tile size
BATCH = 4096
IN_DIM = 2048
HID = 2048
B = BATCH // N_CORES          # 512, batch per core = matmul free dim
KI = IN_DIM // P              # 16, k-tiles for x contraction
KH = HID // P                 # 16, k-tiles for h/c contraction
MT = HID // P                 # 16, output h-tiles

W_NAMES = ["ii", "hi", "if_", "hf", "cf", "ic", "hc", "io", "ho", "co"]


def _build(p, ki, kh, mt, b):
    import concourse.tile as tile
    from concourse import bacc, mybir

    bf16, f32 = mybir.dt.bfloat16, mybir.dt.float32
    Sig = mybir.ActivationFunctionType.Sigmoid
    Tanh = mybir.ActivationFunctionType.Tanh
    Mult = mybir.AluOpType.mult

    nc = bacc.Bacc(
        "TRN2",
        target_bir_lowering=False,
        debug=False,
        num_devices=N_CORES,
    )

    xT = nc.dram_tensor("xT", [p, ki, b], bf16, kind="ExternalInput").ap()
    hT = nc.dram_tensor("hT", [p, kh, b], bf16, kind="ExternalInput").ap()
    cT = nc.dram_tensor("cT", [p, kh, b], bf16, kind="ExternalInput").ap()
    c0T = nc.dram_tensor("c0T", [p, mt, b], f32, kind="ExternalInput").ap()
    bias = nc.dram_tensor("bias", [p, mt, 4], f32, kind="ExternalInput").ap()
    w = {
        n: nc.dram_tensor(
            f"w_{n}", [mt, p, (ki if n in ("ii", "if_", "ic", "io") else kh), p],
            bf16, kind="ExternalInput",
        ).ap()
        for n in W_NAMES
    }
    ogT = nc.dram_tensor("ogT", [p, mt, b], f32, kind="ExternalOutput").ap()
    h1T = nc.dram_tensor("h1T", [p, mt, b], f32, kind="ExternalOutput").ap()
    c1T = nc.dram_tensor("c1T", [p, mt, b], f32, kind="ExternalOutput").ap()

    with tile.TileContext(nc) as tc, ExitStack() as ctx:
        acts = ctx.enter_context(tc.tile_pool(name="acts", bufs=1))
        wpool = ctx.enter_context(tc.tile_pool(name="w", bufs=2))
        cpool = ctx.enter_context(tc.tile_pool(name="c0", bufs=2))
        tpool = ctx.enter_context(tc.tile_pool(name="temps", bufs=2))
        ppool = ctx.enter_context(tc.tile_pool(name="psum", bufs=8, space="PSUM"))

        # resident tensors. Activation loads go on gpsimd (a second DMA issue
        # queue) and are split into chunks so the first matmuls — which only
        # need the first x chunks plus one weight slab — start ~20us earlier
        # than one monolithic 6MB preload would allow.
        CH = 4  # k-tiles per DMA chunk
        xT_sb = acts.tile([p, ki, b], bf16, tag="xT")
        hT_sb = acts.tile([p, kh, b], bf16, tag="hT")
        cT_sb = acts.tile([p, kh, b], bf16, tag="cT")
        for src, dst, nk, eng in ((xT, xT_sb, ki, nc.gpsimd),
                                  (hT, hT_sb, kh, nc.sync),
                                  (cT, cT_sb, kh, nc.gpsimd)):
            ch = min(CH, nk)
            for c in range(0, nk, ch):
                eng.dma_start(dst[:, c:c + ch, :], src[:, c:c + ch, :])
        bias_sb = acts.tile([p, mt, 4], f32, tag="bias")
        nc.gpsimd.dma_start(bias_sb[:], bias[:])
        c1f_sb = acts.tile([p, mt, b], f32, tag="c1f")    # new cell state, fp32
        c1b_sb = acts.tile([p, mt, b], bf16, tag="c1b")   # bf16 copy for o-gate matmul

        def load_w(name, tag, m, chunks=1, eng=None):
            nk = w[name].shape[2]
            t = wpool.tile([p, nk, p], bf16, tag=tag)
            step = max(1, nk // chunks)
            for c in range(0, nk, step):
                (eng or nc.sync).dma_start(t[:, c:c + step], w[name][m, :, c:c + step])
            return t

        def accum(ps, w_t, act_sb, nk, first, last):
            for ko in range(nk):
                nc.tensor.matmul(
                    ps[:], lhsT=w_t[:, ko], rhs=act_sb[:, ko],
                    start=(first and ko == 0), stop=(last and ko == nk - 1),
                )

        # ---- phase 1: i/f/g gates + new cell state ----
        # x-term weights load (and matmul) first so the first m-tile's PE work
        # starts as soon as xT chunks land, while hT/cT still stream in.
        for m in range(mt):
            # m=0/m=1 slab issues go on the otherwise-idle scalar/vector
            # engines: the sync/gpsimd queues take ~650ns per dma_start, so
            # serializing ~30 early descriptors on two engines would delay the
            # DMA ramp by ~10us. Scalar/vector do no work before ~37us.
            first = 4 if m == 0 else 1
            rest = 2 if m < 2 else 1
            eng = nc.scalar if m == 0 else None
            w_ii = load_w("ii", "w0", m, chunks=first, eng=eng)
            w_if = load_w("if_", "w2", m, chunks=first, eng=eng)
            w_ic = load_w("ic", "w5", m, chunks=first, eng=eng)
            w_hi = load_w("hi", "w1", m, chunks=rest, eng=eng)
            w_hf = load_w("hf", "w3", m, chunks=rest, eng=eng)
            w_hc = load_w("hc", "w6", m, chunks=rest, eng=eng)
            w_cf = load_w("cf", "w4", m, chunks=rest, eng=eng)

            ps_i = ppool.tile([p, b], f32, tag="ps")
            ps_f = ppool.tile([p, b], f32, tag="ps")
            ps_g = ppool.tile([p, b], f32, tag="ps")
            accum(ps_i, w_ii, xT_sb, ki, True, False)
            accum(ps_f, w_if, xT_sb, ki, True, False)
            accum(ps_g, w_ic, xT_sb, ki, True, False)
            accum(ps_i, w_hi, hT_sb, kh, False, True)
            accum(ps_f, w_hf, hT_sb, kh, False, False)
            accum(ps_g, w_hc, hT_sb, kh, False, True)
            accum(ps_f, w_cf, cT_sb, kh, False, True)

            i_act = tpool.tile([p, b], f32, tag="i_act")
            nc.scalar.activation(i_act[:], ps_i[:], Sig, bias=bias_sb[:, m, 0:1])
            f_act = tpool.tile([p, b], f32, tag="f_act")
            nc.scalar.activation(f_act[:], ps_f[:], Sig, bias=bias_sb[:, m, 1:2])
            g_act = tpool.tile([p, b], f32, tag="g_act")
            nc.scalar.activation(g_act[:], ps_g[:], Tanh, bias=bias_sb[:, m, 2:3])

            c0_t = cpool.tile([p, b], f32, tag="c0")
            nc.gpsimd.dma_start(c0_t[:], c0T[:, m, :])

            t1 = tpool.tile([p, b], f32, tag="t1")
            nc.vector.tensor_mul(t1[:], f_act[:], c0_t[:])
            nc.vector.tensor_mul(i_act[:], i_act[:], g_act[:])
            c1_m = c1f_sb[:, m, :]
            nc.vector.tensor_add(c1_m, t1[:], i_act[:])
            nc.vector.tensor_copy(out=c1b_sb[:, m, :], in_=c1_m)
            nc.sync.dma_start(c1T[:, m, :], c1_m)

        # ---- phase 2: o gate + h1 ----
        for m in range(mt):
            w_io = load_w("io", "w0", m)
            w_ho = load_w("ho", "w1", m)
            w_co = load_w("co", "w2", m)

            ps_o = ppool.tile([p, b], f32, tag="ps")
            accum(ps_o, w_io, xT_sb, ki, True, False)
            accum(ps_o, w_ho, hT_sb, kh, False, False)
            accum(ps_o, w_co, c1b_sb, kh, False, True)

            o_act = tpool.tile([p, b], f32, tag="o_act")
            nc.scalar.activation(o_act[:], ps_o[:], Sig, bias=bias_sb[:, m, 3:4])
            tc1 = tpool.tile([p, b], f32, tag="tc1")
            nc.scalar.activation(tc1[:], c1f_sb[:, m, :], Tanh)
            h1_t = tpool.tile([p, b], f32, tag="h1")
            nc.vector.tensor_mul(h1_t[:], o_act[:], tc1[:])

            nc.sync.dma_start(ogT[:, m, :], o_act[:])
            nc.sync.dma_start(h1T[:, m, :], h1_t[:])

    nc.compile()
    return nc


_NC = None


def _get_nc():
    global _NC
    if _NC is None:
        _NC = _build(P, KI, KH, MT, B)
    return _NC


# ---------------- host-side packing ----------------

def _pack_actT(a, dtype):
    """(b, d) -> (128, d//128, b) with [ki, ko, b] = a[b, ko*128+ki]."""
    b, d = a.shape
    return np.ascontiguousarray(
        a.T.reshape(d // P, P, b).transpose(1, 0, 2)
    ).astype(dtype, copy=False)


def _pack_w(W):
    """(H, K) -> (H//128, 128, K//128, 128) with [mt, ki, ko, m] = W[mt*128+m, ko*128+ki]."""
    H, K = W.shape
    return np.ascontiguousarray(
        W.reshape(H // P, P, K // P, P).transpose(0, 3, 2, 1).astype(BF)
    )


def _unpack_out(o):
    """(128, mt, b) [p, m, b] -> (b, mt*128)."""
    p, m, b = o.shape
    return np.ascontiguousarray(o.transpose(2, 1, 0).reshape(b, m * p))


def kernel(x, h0, c0,
           W_ii, b_ii, W_hi, b_hi, W_if_, b_if_, W_hf, b_hf, W_cf, b_cf,
           W_ic, b_ic, W_hc, b_hc, W_io, b_io, W_ho, b_ho, W_co, b_co,
           _trace=False):
    from concourse.bass_utils import run_bass_kernel_spmd

    nc = _get_nc()

    x = np.asarray(x, dtype=np.float32)
    h0 = np.asarray(h0, dtype=np.float32)
    c0 = np.asarray(c0, dtype=np.float32)
    (W_ii, W_hi, W_if_, W_hf, W_cf, W_ic, W_hc, W_io, W_ho, W_co) = [
        np.asarray(a, dtype=np.float32)
        for a in (W_ii, W_hi, W_if_, W_hf, W_cf, W_ic, W_hc, W_io, W_ho, W_co)
    ]
    (b_ii, b_hi, b_if_, b_hf, b_cf, b_ic, b_hc, b_io, b_ho, b_co) = [
        np.asarray(a, dtype=np.float32)
        for a in (b_ii, b_hi, b_if_, b_hf, b_cf, b_ic, b_hc, b_io, b_ho, b_co)
    ]

    # combined per-gate biases, packed [p, mt, gate]
    bias = np.stack(
        [
            (b_ii + b_hi).reshape(MT, P).T,
            (b_if_ + b_hf + b_cf).reshape(MT, P).T,
            (b_ic + b_hc).reshape(MT, P).T,
            (b_io + b_ho + b_co).reshape(MT, P).T,
        ],
        axis=2,
    ).astype(np.float32)
    w_packed = {
        f"w_{n}": _pack_w(W)
        for n, W in zip(W_NAMES, [W_ii, W_hi, W_if_, W_hf, W_cf,
                                  W_ic, W_hc, W_io, W_ho, W_co])
    }

    in_maps = []
    for core in range(N_CORES):
        s = slice(core * B, (core + 1) * B)
        m = {
            "xT": _pack_actT(x[s], BF),
            "hT": _pack_actT(h0[s], BF),
            "cT": _pack_actT(c0[s], BF),
            "c0T": _pack_actT(c0[s], np.float32),
            "bias": bias,
        }
        m.update(w_packed)
        in_maps.append(m)

    res = run_bass_kernel_spmd(nc, in_maps, list(range(N_CORES)), trace=_trace)

    o_g = np.empty((BATCH, HID), np.float32)
    h1 = np.empty((BATCH, HID), np.float32)
    c1 = np.empty((BATCH, HID), np.float32)
    for core in range(N_CORES):
        s = slice(core * B, (core + 1) * B)
        o_g[s] = _unpack_out(res.results[core]["ogT"])
        h1[s] = _unpack_out(res.results[core]["h1T"])
        c1[s] = _unpack_out(res.results[core]["c1T"])
    out = (o_g, h1, c1)
    if _trace:
        return out, res
    return out



# revision 2
# speedup vs baseline: 1.4620x; 1.4620x over previous
"""Trainium2 Bass kernel for the nn_LSTMCell problem.

Strategy: data-parallel over the batch dim (4096 -> 8 cores x 512), weights
replicated. All on-chip compute happens in "transposed" orientation
(hidden on PSUM partitions, batch on the free dim) so every matmul operand
can be DMA'd in its natural, contiguous layout:

    gate.T[h, b] = sum_k W.T[k, h] * act.T[k, b]

Mixed precision: 7 of the 10 weight paths (ii, hi, if_, hf, cf, io, co) run
as fp8(e4m3) matmuls in DoubleRow perf mode -- the PE array virtualizes to
128x256, processing two contraction rows per cycle, ~1.5-1.8x the bf16 MM
rate. The 3 error-critical paths stay bf16: the cell-candidate pair (ic, hc),
whose tanh feeds c1 additively, and ho. (Empirically, all-fp8 lands at
norm-rel 2.7e-2 > the 2e-2 gate; this split lands ~1.6e-2 in exact numpy
simulation of the TRN quantization.)

All weights are pre-scaled by 64 on the host (keeps e4m3 weights out of the
subnormal range; exact in bf16), and every gate activation descales with
scale=1/64. fp32 PSUM accumulation; elementwise math and outputs fp32.

DoubleRow operand layout (matches bass_interp semantics
  out += sum_i w[:, i].T @ a[:, i], i in {0, 1}):
  lhsT = w_tile[:, 2t:2t+2]   ->  [K=128, 2, M=128]  fp8
  rhs  = actT [:, 2t:2t+2, :] ->  [K=128, 2, N=512]  fp8
pairing slot i with contraction index k = (2t+i)*128 + p, which is exactly
the existing [p, ktile, b] / [mt, p, ktile, m] packed layouts -- so fp8
tensors reuse the bf16 packing, only the dtype changes.

Per core:
  phase 1: for each of 16 h-tiles: i/f gates via fp8 DR (40 MMs), g gate via
           bf16 (32 MMs), sigmoid/tanh, c1 = f*c0 + i*tanh(g) (fp32, kept in
           SBUF + DMA'd out), c1 cast to fp8 for the o-gate's co matmul.
  phase 2: for each of 16 h-tiles: o gate = io(fp8 DR, 8) + ho(bf16, 16) +
           co(fp8 DR, 8), o = sigmoid(...), h1 = o * tanh(c1), DMA out.
"""

import numpy as np
import ml_dtypes
from contextlib import ExitStack

BF = ml_dtypes.bfloat16
E4 = ml_dtypes.float8_e4m3   # TRN fp8_e4m3: max normal 240, matches device cast
WSCALE = 64.0

N_CORES = 8
P = 128          # partition dim / k-tile size / m-tile size
BATCH = 4096
IN_DIM = 2048
HID = 2048
B = BATCH // N_CORES          # 512, batch per core = matmul free dim
KI = IN_DIM // P              # 16, k-tiles for x contraction
KH = HID // P                 # 16, k-tiles for h/c contraction
MT = HID // P                 # 16, output h-tiles

FP8_PATHS = ("ii", "hi", "if_", "hf", "cf", "io", "co")
BF16_PATHS = ("ic", "hc", "ho")
W_NAMES = ["ii", "hi", "if_", "hf", "cf", "ic", "hc", "io", "ho", "co"]


def _build(p, ki, kh, mt, b):
    import concourse.tile as tile
    from concourse import bacc, mybir

    bf16, f32 = mybir.dt.bfloat16, mybir.dt.float32
    fp8 = mybir.dt.float8e4
    Sig = mybir.ActivationFunctionType.Sigmoid
    Tanh = mybir.ActivationFunctionType.Tanh
    DR = mybir.MatmulPerfMode.DoubleRow
    INV = 1.0 / WSCALE

    nc = bacc.Bacc(
        "TRN2",
        target_bir_lowering=False,
        debug=False,
        num_devices=N_CORES,
    )

    xT8 = nc.dram_tensor("xT8", [p, ki, b], fp8, kind="ExternalInput").ap()
    hT8 = nc.dram_tensor("hT8", [p, kh, b], fp8, kind="ExternalInput").ap()
    cT8 = nc.dram_tensor("cT8", [p, kh, b], fp8, kind="ExternalInput").ap()
    xTb = nc.dram_tensor("xTb", [p, ki, b], bf16, kind="ExternalInput").ap()
    hTb = nc.dram_tensor("hTb", [p, kh, b], bf16, kind="ExternalInput").ap()
    c0T = nc.dram_tensor("c0T", [p, mt, b], f32, kind="ExternalInput").ap()
    bias = nc.dram_tensor("bias", [p, mt, 4], f32, kind="ExternalInput").ap()
    w = {
        n: nc.dram_tensor(
            f"w_{n}", [mt, p, (ki if n in ("ii", "if_", "ic", "io") else kh), p],
            (fp8 if n in FP8_PATHS else bf16), kind="ExternalInput",
        ).ap()
        for n in W_NAMES
    }
    ogT = nc.dram_tensor("ogT", [p, mt, b], f32, kind="ExternalOutput").ap()
    h1T = nc.dram_tensor("h1T", [p, mt, b], f32, kind="ExternalOutput").ap()
    c1T = nc.dram_tensor("c1T", [p, mt, b], f32, kind="ExternalOutput").ap()

    with tile.TileContext(nc) as tc, ExitStack() as ctx:
        acts = ctx.enter_context(tc.tile_pool(name="acts", bufs=1))
        wpool = ctx.enter_context(tc.tile_pool(name="w", bufs=2))
        cpool = ctx.enter_context(tc.tile_pool(name="c0", bufs=2))
        tpool = ctx.enter_context(tc.tile_pool(name="temps", bufs=2))
        ppool = ctx.enter_context(tc.tile_pool(name="psum", bufs=8, space="PSUM"))

        # resident activations. fp8 x lands first (x-gates run first), the
        # bf16 copies for the g gate next, then h/c. Chunked so the first
        # matmuls only wait on their own chunks.
        CH = 4  # k-tiles per DMA chunk
        xT8_sb = acts.tile([p, ki, b], fp8, tag="xT8")
        xTb_sb = acts.tile([p, ki, b], bf16, tag="xTb")
        hT8_sb = acts.tile([p, kh, b], fp8, tag="hT8")
        hTb_sb = acts.tile([p, kh, b], bf16, tag="hTb")
        cT8_sb = acts.tile([p, kh, b], fp8, tag="cT8")
        for src, dst, nk, eng in ((xT8, xT8_sb, ki, nc.gpsimd),
                                  (xTb, xTb_sb, ki, nc.sync),
                                  (hT8, hT8_sb, kh, nc.gpsimd),
                                  (hTb, hTb_sb, kh, nc.sync),
                                  (cT8, cT8_sb, kh, nc.gpsimd)):
            ch = min(CH, nk)
            for c in range(0, nk, ch):
                eng.dma_start(dst[:, c:c + ch, :], src[:, c:c + ch, :])
        bias_sb = acts.tile([p, mt, 4], f32, tag="bias")
        nc.gpsimd.dma_start(bias_sb[:], bias[:])
        c1f_sb = acts.tile([p, mt, b], f32, tag="c1f")   # new cell state, fp32
        c18_sb = acts.tile([p, mt, b], fp8, tag="c18")   # fp8 copy for co matmul

        def load_w(name, tag, m, chunks=1, eng=None):
            nk = w[name].shape[2]
            dt = fp8 if name in FP8_PATHS else bf16
            t = wpool.tile([p, nk, p], dt, tag=tag)
            step = max(1, nk // chunks)
            for c in range(0, nk, step):
                (eng or nc.sync).dma_start(t[:, c:c + step], w[name][m, :, c:c + step])
            return t

        def accum8(ps, w_t, act_sb, nk, first, last):
            # fp8 DoubleRow: each MM consumes 2 k-tiles (K=256 virtual)
            for t in range(nk // 2):
                nc.tensor.matmul(
                    ps[:], lhsT=w_t[:, 2 * t:2 * t + 2],
                    rhs=act_sb[:, 2 * t:2 * t + 2, :],
                    start=(first and t == 0), stop=(last and t == nk // 2 - 1),
                    perf_mode=DR,
                )

        def accumb(ps, w_t, act_sb, nk, first, last):
            for ko in range(nk):
                nc.tensor.matmul(
                    ps[:], lhsT=w_t[:, ko], rhs=act_sb[:, ko],
                    start=(first and ko == 0), stop=(last and ko == nk - 1),
                )

        # ---- phase 1: i/f/g gates + new cell state ----
        for m in range(mt):
            # m=0/m=1 slab issues go on the otherwise-idle scalar/vector
            # engines so the sync/gpsimd queues (busy with the activation
            # preload) don't delay the DMA ramp.
            first = 4 if m == 0 else 1
            rest = 2 if m < 2 else 1
            eng = nc.scalar if m == 0 else None
            w_ii = load_w("ii", "w0", m, chunks=first, eng=eng)
            w_if = load_w("if_", "w2", m, chunks=first, eng=eng)
            w_ic = load_w("ic", "w5", m, chunks=first, eng=eng)
            w_hi = load_w("hi", "w1", m, chunks=rest, eng=eng)
            w_hf = load_w("hf", "w3", m, chunks=rest, eng=eng)
            w_hc = load_w("hc", "w6", m, chunks=rest, eng=eng)
            w_cf = load_w("cf", "w4", m, chunks=rest, eng=eng)

            ps_i = ppool.tile([p, b], f32, tag="ps")
            ps_f = ppool.tile([p, b], f32, tag="ps")
            ps_g = ppool.tile([p, b], f32, tag="ps")
            accum8(ps_i, w_ii, xT8_sb, ki, True, False)
            accum8(ps_f, w_if, xT8_sb, ki, True, False)
            accumb(ps_g, w_ic, xTb_sb, ki, True, False)
            accum8(ps_i, w_hi, hT8_sb, kh, False, True)
            accum8(ps_f, w_hf, hT8_sb, kh, False, False)
            accumb(ps_g, w_hc, hTb_sb, kh, False, True)
            accum8(ps_f, w_cf, cT8_sb, kh, False, True)

            i_act = tpool.tile([p, b], f32, tag="i_act")
            nc.scalar.activation(i_act[:], ps_i[:], Sig,
                                 bias=bias_sb[:, m, 0:1], scale=INV)
            f_act = tpool.tile([p, b], f32, tag="f_act")
            nc.scalar.activation(f_act[:], ps_f[:], Sig,
                                 bias=bias_sb[:, m, 1:2], scale=INV)
            g_act = tpool.tile([p, b], f32, tag="g_act")
            nc.scalar.activation(g_act[:], ps_g[:], Tanh,
                                 bias=bias_sb[:, m, 2:3], scale=INV)

            c0_t = cpool.tile([p, b], f32, tag="c0")
            nc.gpsimd.dma_start(c0_t[:], c0T[:, m, :])

            t1 = tpool.tile([p, b], f32, tag="t1")
            nc.vector.tensor_mul(t1[:], f_act[:], c0_t[:])
            nc.vector.tensor_mul(i_act[:], i_act[:], g_act[:])
            c1_m = c1f_sb[:, m, :]
            nc.vector.tensor_add(c1_m, t1[:], i_act[:])
            nc.vector.tensor_copy(out=c18_sb[:, m, :], in_=c1_m)
            nc.sync.dma_start(c1T[:, m, :], c1_m)

        # ---- phase 2: o gate + h1 ----
        for m in range(mt):
            w_io = load_w("io", "w0", m)
            w_ho = load_w("ho", "w1", m)
            w_co = load_w("co", "w2", m)

            ps_o = ppool.tile([p, b], f32, tag="ps")
            accum8(ps_o, w_io, xT8_sb, ki, True, False)
            accumb(ps_o, w_ho, hTb_sb, kh, False, False)
            accum8(ps_o, w_co, c18_sb, kh, False, True)

            o_act = tpool.tile([p, b], f32, tag="o_act")
            nc.scalar.activation(o_act[:], ps_o[:], Sig,
                                 bias=bias_sb[:, m, 3:4], scale=INV)
            tc1 = tpool.tile([p, b], f32, tag="tc1")
            nc.scalar.activation(tc1[:], c1f_sb[:, m, :], Tanh)
            h1_t = tpool.tile([p, b], f32, tag="h1")
            nc.vector.tensor_mul(h1_t[:], o_act[:], tc1[:])

            nc.sync.dma_start(ogT[:, m, :], o_act[:])
            nc.sync.dma_start(h1T[:, m, :], h1_t[:])

    nc.compile()
    return nc


_NC = None


def _get_nc():
    global _NC
    if _NC is None:
        _NC = _build(P, KI, KH, MT, B)
    return _NC


# ---------------- host-side packing ----------------

def _pack_actT(a, dtype):
    """(b, d) -> (128, d//128, b) with [ki, ko, b] = a[b, ko*128+ki]."""
    b, d = a.shape
    return np.ascontiguousarray(
        a.T.reshape(d // P, P, b).transpose(1, 0, 2)
    ).astype(dtype, copy=False)


def _pack_w(W, dtype):
    """(H, K) -> (H//128, 128, K//128, 128), [mt, ki, ko, m] = 64*W[mt*128+m, ko*128+ki]."""
    H, K = W.shape
    return np.ascontiguousarray(
        (W.reshape(H // P, P, K // P, P).transpose(0, 3, 2, 1) * WSCALE).astype(dtype)
    )


def _unpack_out(o):
    """(128, mt, b) [p, m, b] -> (b, mt*128)."""
    p, m, b = o.shape
    return np.ascontiguousarray(o.transpose(2, 1, 0).reshape(b, m * p))


def kernel(x, h0, c0,
           W_ii, b_ii, W_hi, b_hi, W_if_, b_if_, W_hf, b_hf, W_cf, b_cf,
           W_ic, b_ic, W_hc, b_hc, W_io, b_io, W_ho, b_ho, W_co, b_co,
           _trace=False):
    from concourse.bass_utils import run_bass_kernel_spmd

    nc = _get_nc()

    x = np.asarray(x, dtype=np.float32)
    h0 = np.asarray(h0, dtype=np.float32)
    c0 = np.asarray(c0, dtype=np.float32)
    Ws = {n: np.asarray(a, dtype=np.float32)
          for n, a in zip(W_NAMES, (W_ii, W_hi, W_if_, W_hf, W_cf,
                                    W_ic, W_hc, W_io, W_ho, W_co))}
    (b_ii, b_hi, b_if_, b_hf, b_cf, b_ic, b_hc, b_io, b_ho, b_co) = [
        np.asarray(a, dtype=np.float32)
        for a in (b_ii, b_hi, b_if_, b_hf, b_cf, b_ic, b_hc, b_io, b_ho, b_co)
    ]

    # combined per-gate biases, packed [p, mt, gate]
    bias = np.stack(
        [
            (b_ii + b_hi).reshape(MT, P).T,
            (b_if_ + b_hf + b_cf).reshape(MT, P).T,
            (b_ic + b_hc).reshape(MT, P).T,
            (b_io + b_ho + b_co).reshape(MT, P).T,
        ],
        axis=2,
    ).astype(np.float32)
    w_packed = {
        f"w_{n}": _pack_w(W, E4 if n in FP8_PATHS else BF)
        for n, W in Ws.items()
    }

    in_maps = []
    for core in range(N_CORES):
        s = slice(core * B, (core + 1) * B)
        m = {
            "xT8": _pack_actT(x[s], E4),
            "xTb": _pack_actT(x[s], BF),
            "hT8": _pack_actT(h0[s], E4),
            "hTb": _pack_actT(h0[s], BF),
            "cT8": _pack_actT(c0[s], E4),
            "c0T": _pack_actT(c0[s], np.float32),
            "bias": bias,
        }
        m.update(w_packed)
        in_maps.append(m)

    res = run_bass_kernel_spmd(nc, in_maps, list(range(N_CORES)), trace=_trace)

    o_g = np.empty((BATCH, HID), np.float32)
    h1 = np.empty((BATCH, HID), np.float32)
    c1 = np.empty((BATCH, HID), np.float32)
    for core in range(N_CORES):
        s = slice(core * B, (core + 1) * B)
        o_g[s] = _unpack_out(res.results[core]["ogT"])
        h1[s] = _unpack_out(res.results[core]["h1T"])
        c1[s] = _unpack_out(res.results[core]["c1T"])
    out = (o_g, h1, c1)
    if _trace:
        return out, res
    return out
